# revision 31
# baseline (speedup 1.0000x reference)
"""Trainium2 Bass kernel for nn_Net_67954972557347 (dense_mlp).

Network: a1 = lrelu(a@Wa+ba) [B,68]; b1 = lrelu(b@Wb+bb) [B,68];
c = [a1|b1|meta] [B,140]; then 10 lrelu'd dense layers
(140->34->34->20->20->20->20->20->5->2->1), lrelu slope 0.01.

Strategy: pure data parallel over 8 cores (32768 rows each), activations
feature-major ([feat, batch]); each layer is a PE matmul with the batch
streaming as the moving operand. All matmul tensors are FP16: fp32r
streams rows at only ~1.2 G rows/s on the PE while fp16 runs the full
1 row/cycle @ 2.4 GHz, and fp16 halves the HBM traffic; PSUM stays
fp32. meta rides as a +/- pair (m = (lrelu(m)-lrelu(-m))/1.01, folded
into the next layer's weights) so no large-magnitude inverse-lrelu
values hit fp16.

Host packs two multiple-of-8-partition fp16 input streams (odd
partition counts like 103 collapse HWDGE DMA to ~25 GB/s because SDMA
engines are partition-bound; 56/104 spread acceptably and fp16 halves
the bytes). ALL biases are folded into matmuls via ones rows, so every
drain is a zero-bias leaky-relu:
  t_a [ 56, 32768]: rows 0:45 a.T, 45:49 meta.T, 49:53 -meta.T,
                    53 ones, 54:56 zero pad
  t_b [104, 32768]: rows 0:102 b.T, 102 ones, 103 zero pad

Per 512-column chunk, 7 matmul passes into two PSUM tiles: [128,1536]
(al | X | Y -- one tile so the merged ACT drain reads contiguously) and
a separate [128,512] for be (own tile so its DVE reader's WAR does not
falsely gate the other banks -- pool WAR tracking is whole-tile).
Cross-read distances are the minimum each pass's position in the step
allows (P5 is the first matmul and needs 3; the rest ride at 2), giving
PIPE = 2+3+2+7x2 = 21:
  P5 axy(t-3).X -> X[0:109]  = [0; c1=W1.c0+B1; ones]      (start)
  P3 axy(t-2).al-> X[0:34]  += W0.c-part(a1,meta+/-)+B0
  P4 bs (t-2)   -> X[0:34]  += W0[68:136].b1               (stop)
  P7 axy(t-2).Y -> Y[0:109]  = tail adv c2->c3,...,c8->y,
                               +B3..B9, ones               (start)
  P6 axy(t-2).X -> Y[0:21]  += W2.c1 + B2                  (stop)
  P1 t_a[0:54]  -> al[0:109] = [a1-pre; m+; m-; ones; 0]   (+ba)
  P2 t_b[0:103] -> be[0:68]  = b1-pre                      (+bb)
Drains (3 ops, PSUM reads are single-operand only - NCC_IBVF027):
  ACT: ONE zero-bias Prelu over the contiguous al|X|Y banks ->
       axy [128,1536] fp16 (shrinks to X|Y then Y as the pipe drains)
  DVE: CAST be->bs (fp16) then in-SBUF stt lrelu.
X layout [c0(0:34); c1(34:68); ones(68)]; Y layout [y(0); c2(1:21);
c3(21:41); c4(41:61); c5(61:81); c6(81:101); c7(101:106); c8(106:108);
ones(108)]. y(chunk) lands at step chunk+PIPE, PIPE=21. Inputs stream
as 4-chunk super-DMAs. Every pass is a uniform (128,128) PE tile
(mixed tile_size configs block weight-load overlap and pin the clock
at the 1.2 GHz p-state; uniform tiles sustain 2.4 GHz, 215ns/pass).
"""

import os
import sys

import numpy as np

for _p in ("/opt/trn_rl_repo", "/root/.axon_site/_ro/trn_rl_repo"):
    if os.path.isdir(_p) and _p not in sys.path:
        sys.path.append(_p)

import concourse.bass as bass
import concourse.mybir as mybir
import concourse.tile as tile
from concourse import bacc
from concourse.bass_utils import run_bass_kernel_spmd
from bass_rust import add_dep_helper

F32 = mybir.dt.float32
F16 = mybir.dt.float16
ALU = mybir.AluOpType
PRELU = mybir.ActivationFunctionType.Prelu

B_FULL = 262144
N_CORES = 8
B_CORE = B_FULL // N_CORES          # 32768
N = 512                              # columns per chunk (PSUM bank, fp32)
SUP = 4                              # chunks per t1 super-DMA / t2 group
PIPE = 20                            # all read distances 2: 2+2+2+7x2
ALPHA = 0.01                         # leaky-relu slope
RM = 1.0 / 1.01                      # +/- meta reconstruction factor

MW = 128                             # uniform M: every pass runs a full
KA, KB = 128, 128                    # (128,128) PE tile so the weight-load
# weight-tile column spans             pipeline config never changes
C1, C2, C3, C4, C5, C6, C7 = 0, 128, 256, 384, 512, 640, 768
WT_COLS = 1024


def _pack_weights(Wa, ba, Wb, bb, Ws, Bs):
    """Build the [128, WT_COLS] packed fp16 weight tile."""
    W0, W1, W2, W3, W4, W5, W6, W7, W8, W9 = Ws
    B0, B1, B2, B3, B4, B5, B6, B7, B8, B9 = Bs
    wt = np.zeros((128, WT_COLS), np.float32)
    # P1: rhs t1[0:54] -> al: a1-pre(0:68), m+(68:72), m-(72:76), ones(76)
    wt[0:45, C1:C1 + 68] = Wa
    wt[45:49, C1 + 68:C1 + 72] = np.eye(4, dtype=np.float32)
    wt[49:53, C1 + 72:C1 + 76] = np.eye(4, dtype=np.float32)
    wt[53, C1:C1 + 68] = ba
    wt[53, C1 + 76] = 1.0
    # P2: rhs t_b[0:103] -> be: b1-pre + bb via ones
    wt[0:102, C2:C2 + 68] = Wb
    wt[102, C2:C2 + 68] = bb
    # P3: rhs axy.al[0:77] -> X[0:34]: c0 partial + B0
    # meta = (lrelu(m) - lrelu(-m)) / 1.01 folded into W0[136:140]
    wt[0:68, C3:C3 + 34] = W0[0:68]
    wt[68:72, C3:C3 + 34] = W0[136:140] * RM
    wt[72:76, C3:C3 + 34] = -W0[136:140] * RM
    wt[76, C3:C3 + 34] = B0
    # P4: rhs bs[0:68] -> X[0:34]: c0 partial
    wt[0:68, C4:C4 + 34] = W0[68:136]
    # P5: rhs axy.X[0:69] -> X[0:109]: c1 = W1.c0 + B1 (34:68), ones(68)
    wt[0:34, C5 + 34:C5 + 68] = W1
    wt[68, C5 + 34:C5 + 68] = B1
    wt[68, C5 + 68] = 1.0
    # P6: rhs axy.X[0:69] -> Y[0:21]: c2 = W2.c1 + B2 (cols 1:21)
    wt[34:68, C6 + 1:C6 + 21] = W2
    wt[68, C6 + 1:C6 + 21] = B2
    # P7: rhs axy.Y[0:109] -> Y[0:109]: tail chain + biases + ones
    wt[1:21, C7 + 21:C7 + 41] = W3       # c2 -> c3
    wt[21:41, C7 + 41:C7 + 61] = W4      # c3 -> c4
    wt[41:61, C7 + 61:C7 + 81] = W5      # c4 -> c5
    wt[61:81, C7 + 81:C7 + 101] = W6     # c5 -> c6
    wt[81:101, C7 + 101:C7 + 106] = W7   # c6 -> c7
    wt[101:106, C7 + 106:C7 + 108] = W8  # c7 -> c8
    wt[106:108, C7:C7 + 1] = W9          # c8 -> y
    wt[108, C7 + 21:C7 + 41] = B3
    wt[108, C7 + 41:C7 + 61] = B4
    wt[108, C7 + 61:C7 + 81] = B5
    wt[108, C7 + 81:C7 + 101] = B6
    wt[108, C7 + 101:C7 + 106] = B7
    wt[108, C7 + 106:C7 + 108] = B8
    wt[108, C7:C7 + 1] = B9
    wt[108, C7 + 108] = 1.0
    return wt.astype(np.float16)


def _pack_core_inputs(a, b, meta, n_chunks):
    """Pack one core's shard into the fp16 t_a/t_b DMA streams."""
    bc = n_chunks * N
    t_a = np.zeros((KA, bc), np.float16)
    t_a[0:45] = a[:bc].T
    t_a[45:49] = meta[:bc].T
    t_a[49:53] = -meta[:bc].T
    t_a[53] = 1.0
    t_b = np.zeros((KB, bc), np.float16)
    t_b[0:102] = b[:bc].T
    t_b[102] = 1.0
    return t_a, t_b


def build_bass(n_chunks):
    """Build + compile the per-core Bass program (same on all 8 cores)."""
    nc = bacc.Bacc(None, target_bir_lowering=False, debug=False)
    n_steps = n_chunks + PIPE
    n_super = (n_chunks + SUP - 1) // SUP
    NS = SUP * N

    ta_d = nc.dram_tensor("ta", [KA, n_chunks * N], F16,
                          kind="ExternalInput")
    tb_d = nc.dram_tensor("tb", [KB, n_chunks * N], F16,
                          kind="ExternalInput")
    wt_d = nc.dram_tensor("wt", [128, WT_COLS], F16, kind="ExternalInput")
    y_d = nc.dram_tensor("y", [1, n_chunks * N], F16, kind="ExternalOutput")

    with tile.TileContext(nc) as tc:
        with (
            tc.tile_pool(name="const", bufs=1) as constp,
            tc.tile_pool(name="t1p", bufs=3) as t1p,
            tc.tile_pool(name="t2p", bufs=3) as t2p,
            tc.tile_pool(name="axyp", bufs=4) as axyp,
            tc.tile_pool(name="bsp", bufs=4) as bsp,
            tc.tile_pool(name="ps", bufs=2, space=bass.MemorySpace.PSUM) as ps,
            tc.tile_pool(name="psb", bufs=2,
                         space=bass.MemorySpace.PSUM) as psb,
        ):
            wt = constp.tile([128, WT_COLS], F16, tag="wt")
            axy0 = constp.tile([128, 3 * N], F16, tag="axy0")
            nc.sync.dma_start(wt[:], wt_d[:])
            # all-ones init: the warmup lanes only feed garbage chunks
            # (never exported); the ones rows (68/X, 108/Y) must be 1.
            nc.gpsimd.memset(axy0[:], 1.0)

            def w(c0, m):
                return wt[:, c0:c0 + m]

            def chain(*insts):
                for i in range(1, len(insts)):
                    add_dep_helper(insts[i].ins, insts[i - 1].ins,
                                   sync=False, reason="psum acc order")

            t1s, t2s, axys, bss = {}, {}, {}, {}
            axys[-1] = axys[-2] = axys[-3] = axy0

            for t in range(n_steps):
                # ---- input super-DMAs (prefetch 2 supers ahead) ----
                if t % SUP == 0:
                    for s in ([0, 1, 2] if t == 0 else [t // SUP + 2]):
                        if 0 <= s < n_super:
                            t1s[s] = t1p.tile([KA, NS], F16, tag="ta",
                                              name=f"ta_{s}")
                            t2s[s] = t2p.tile([KB, NS], F16, tag="tb",
                                              name=f"tb_{s}")
                            nc.sync.dma_start(
                                t1s[s][:], ta_d[:, s * NS:(s + 1) * NS])
                            nc.sync.dma_start(
                                t2s[s][:], tb_d[:, s * NS:(s + 1) * NS])

                mm = nc.tensor.matmul
                pst = ps.tile([128, 3 * N], F32, tag="ps", name=f"ps_{t}")
                bet = psb.tile([128, N], F32, tag="be", name=f"be_{t}")
                al = pst[:, 0 * N:1 * N]
                X = pst[:, 1 * N:2 * N]
                Y = pst[:, 2 * N:3 * N]
                be = bet[:, 0:N]
                ax2 = axys[t - 2]
                ax3 = axys[t - 3]
                bsp_t = bss.get(t - 2)
                ksl = slice((t % SUP) * N, (t % SUP + 1) * N)

                has_p34 = 2 <= t < n_chunks + 2
                has_p5 = t < n_chunks + 4
                has_p6 = t < n_chunks + 6
                has_p12 = t < n_chunks

                # ---- X bank: P5 (start) <- P3 <- P4 ----
                if has_p5:
                    i5 = mm(X[0:MW], w(C5, MW)[0:128], ax2[0:128, N:2 * N],
                            start=True, stop=not has_p34,
                            tile_position=(0, 0))
                    if has_p34:
                        i3 = mm(X[0:MW], w(C3, MW)[0:128], ax2[0:128, 0:N],
                                start=False, stop=False, tile_position=(0, 0))
                        i4 = mm(X[0:MW], w(C4, MW)[0:128],
                                bsp_t[0:128, 0:N],
                                start=False, stop=True, tile_position=(0, 0))
                        chain(i5, i3, i4)

                # ---- Y bank: P7 (start) <- P6 ----
                i7 = mm(Y[0:MW], w(C7, MW)[0:128], ax2[0:128, 2 * N:3 * N],
                        start=True, stop=not has_p6, tile_position=(0, 0))
                if has_p6:
                    i6 = mm(Y[0:MW], w(C6, MW)[0:128], ax2[0:128, N:2 * N],
                            start=False, stop=True, tile_position=(0, 0))
                    chain(i7, i6)

                # ---- al/be banks: P1, P2a+P2b ----
                if has_p12:
                    s = t // SUP
                    mm(al[0:MW], w(C1, MW)[0:128], t1s[s][0:128, ksl],
                       start=True, stop=True, tile_position=(0, 0))
                    mm(be[0:MW], w(C2, MW)[0:128], t2s[s][0:128, ksl],
                       start=True, stop=True, tile_position=(0, 0))

                # ---- drains ----
                axys[t] = axyp.tile([128, 3 * N], F16, tag="axy",
                                    name=f"axy_{t}")
                lo = 0 if has_p12 else (N if has_p5 else 2 * N)
                nc.scalar.activation(axys[t][0:MW, lo:3 * N],
                                     pst[0:MW, lo:3 * N], PRELU,
                                     bias=0.0, alpha=ALPHA)
                if has_p12:
                    bss[t] = bsp.tile([128, N], F16, tag="bs",
                                      name=f"bs_{t}")
                    nc.vector.tensor_copy(bss[t][0:128, 0:N],
                                          be[0:128])
                    nc.vector.scalar_tensor_tensor(
                        bss[t][0:128, 0:N], bss[t][0:128, 0:N], ALPHA,
                        bss[t][0:128, 0:N], ALU.mult, ALU.max)

                # ---- y out ----
                if t >= PIPE:
                    c = t - PIPE
                    nc.gpsimd.dma_start(
                        y_d[:, c * N:(c + 1) * N],
                        axys[t][0:1, 2 * N:3 * N])

    nc.compile()
    return nc


_NC_CACHE = {}


def _get_nc(n_chunks):
    if n_chunks not in _NC_CACHE:
        _NC_CACHE[n_chunks] = build_bass(n_chunks)
    return _NC_CACHE[n_chunks]


def run_cores(inputs, n_chunks, cores, trace=False, trace_kwargs=None):
    """Pack inputs, run the SPMD kernel on the given cores, return
    (per-core y arrays, BassKernelResults)."""
    a = np.asarray(inputs["a"], np.float32)
    b = np.asarray(inputs["b"], np.float32)
    meta = np.asarray(inputs["meta"], np.float32)
    Ws = [np.asarray(inputs[f"W{i}"], np.float32) for i in range(10)]
    Bs = [np.asarray(inputs[f"B{i}"], np.float32) for i in range(10)]
    wt = _pack_weights(np.asarray(inputs["Wa"], np.float32),
                       np.asarray(inputs["ba"], np.float32),
                       np.asarray(inputs["Wb"], np.float32),
                       np.asarray(inputs["bb"], np.float32), Ws, Bs)
    in_maps = []
    for r in cores:
        sl = slice(r * B_CORE, r * B_CORE + n_chunks * N)
        t_a, t_b = _pack_core_inputs(a[sl], b[sl], meta[sl], n_chunks)
        in_maps.append({"ta": t_a, "tb": t_b, "wt": wt})
    nc = _get_nc(n_chunks)
    kw = dict(trace=trace)
    if trace_kwargs:
        kw.update(trace_kwargs)
    res = run_bass_kernel_spmd(nc, in_maps, list(range(len(cores))), **kw)
    return [res.results[i]["y"].astype(np.float32) for i in range(len(cores))], res


def kernel(**inputs):
    n_chunks = B_CORE // N
    ys, _ = run_cores(inputs, n_chunks, list(range(N_CORES)))
    out = np.empty((B_FULL, 1), np.float32)
    for r in range(N_CORES):
        out[r * B_CORE:(r + 1) * B_CORE, 0] = ys[r][0]
    return out


# revision 32
# speedup vs baseline: 1.0504x; 1.0504x over previous
"""Trainium2 Bass kernel for nn_Net_67954972557347 (dense_mlp).

Network: a1 = lrelu(a@Wa+ba) [B,68]; b1 = lrelu(b@Wb+bb) [B,68];
c = [a1|b1|meta] [B,140]; then 10 lrelu'd dense layers
(140->34->34->20->20->20->20->20->5->2->1), lrelu slope 0.01.

Strategy: pure data parallel over 8 cores (32768 rows each), activations
feature-major ([feat, batch]); each layer is a PE matmul with the batch
streaming as the moving operand. All matmul tensors are FP16: fp32r
streams rows at only ~1.2 G rows/s on the PE while fp16 runs the full
1 row/cycle @ 2.4 GHz, and fp16 halves the HBM traffic; PSUM stays
fp32. meta rides as a +/- pair (m = (lrelu(m)-lrelu(-m))/1.01, folded
into the next layer's weights) so no large-magnitude inverse-lrelu
values hit fp16.

Host packs two multiple-of-8-partition fp16 input streams (odd
partition counts like 103 collapse HWDGE DMA to ~25 GB/s because SDMA
engines are partition-bound; 56/104 spread acceptably and fp16 halves
the bytes). ALL biases are folded into matmuls via ones rows, so every
drain is a zero-bias leaky-relu:
  t_a [ 56, 32768]: rows 0:45 a.T, 45:49 meta.T, 49:53 -meta.T,
                    53 ones, 54:56 zero pad
  t_b [104, 32768]: rows 0:102 b.T, 102 ones, 103 zero pad

Per 512-column chunk, 7 matmul passes into two PSUM tiles: [128,1536]
(al | X | Y -- one tile so the merged ACT drain reads contiguously) and
a separate [128,512] for be (own tile so its DVE reader's WAR does not
falsely gate the other banks -- pool WAR tracking is whole-tile).
All cross-reads ride at distance 2 (P5's RAW edge coincides with the
already-binding ACT->first-writer PSUM WAR edge, so distance 2 adds no
new constraint), giving PIPE = 2+2+2+7x2 = 20:
  P5 axy(t-2).X -> X[0:109]  = [0; c1=W1.c0+B1; ones]      (start)
  P3 axy(t-2).al-> X[0:34]  += W0.c-part(a1,meta+/-)+B0
  P4 bs (t-2)   -> X[0:34]  += W0[68:136].b1               (stop)
  P7 axy(t-2).Y -> Y[0:109]  = tail adv c2->c3,...,c8->y,
                               +B3..B9, ones               (start)
  P6 axy(t-2).X -> Y[0:21]  += W2.c1 + B2                  (stop)
  P1 t_a[0:54]  -> al[0:109] = [a1-pre; m+; m-; ones; 0]   (+ba)
  P2 t_b[0:103] -> be[0:68]  = b1-pre                      (+bb)
Drains (3 ops, PSUM reads are single-operand only - NCC_IBVF027):
  ACT: ONE zero-bias Prelu over the contiguous al|X|Y banks ->
       axy [128,1536] fp16 (shrinks to X|Y then Y as the pipe drains)
  DVE: CAST be->bs (fp16) then in-SBUF stt lrelu.
X layout [c0(0:34); c1(34:68); ones(68)]; Y layout [y(0); c2(1:21);
c3(21:41); c4(41:61); c5(61:81); c6(81:101); c7(101:106); c8(106:108);
ones(108)]. y(chunk) lands at step chunk+PIPE, PIPE=20. Inputs stream
as 4-chunk super-DMAs. Every pass is a uniform (128,128) PE tile
(mixed tile_size configs block weight-load overlap and pin the clock
at the 1.2 GHz p-state; uniform tiles sustain 2.4 GHz, 215ns/pass).
"""

import os
import sys

import numpy as np

for _p in ("/opt/trn_rl_repo", "/root/.axon_site/_ro/trn_rl_repo"):
    if os.path.isdir(_p) and _p not in sys.path:
        sys.path.append(_p)

import concourse.bass as bass
import concourse.mybir as mybir
import concourse.tile as tile
from concourse import bacc
from concourse.bass_utils import run_bass_kernel_spmd
from bass_rust import add_dep_helper

F32 = mybir.dt.float32
F16 = mybir.dt.float16
ALU = mybir.AluOpType
PRELU = mybir.ActivationFunctionType.Prelu

B_FULL = 262144
N_CORES = 8
B_CORE = B_FULL // N_CORES          # 32768
N = 512                              # columns per chunk (PSUM bank, fp32)
SUP = 4                              # chunks per t1 super-DMA / t2 group
PIPE = 20                            # all read distances 2: 2+2+2+7x2
ALPHA = 0.01                         # leaky-relu slope
RM = 1.0 / 1.01                      # +/- meta reconstruction factor

MW = 128                             # uniform M: every pass runs a full
KA, KB = 128, 128                    # (128,128) PE tile so the weight-load
# weight-tile column spans             pipeline config never changes
C1, C2, C3, C4, C5, C6, C7 = 0, 128, 256, 384, 512, 640, 768
WT_COLS = 1024


def _pack_weights(Wa, ba, Wb, bb, Ws, Bs):
    """Build the [128, WT_COLS] packed fp16 weight tile."""
    W0, W1, W2, W3, W4, W5, W6, W7, W8, W9 = Ws
    B0, B1, B2, B3, B4, B5, B6, B7, B8, B9 = Bs
    wt = np.zeros((128, WT_COLS), np.float32)
    # P1: rhs t1[0:54] -> al: a1-pre(0:68), m+(68:72), m-(72:76), ones(76)
    wt[0:45, C1:C1 + 68] = Wa
    wt[45:49, C1 + 68:C1 + 72] = np.eye(4, dtype=np.float32)
    wt[49:53, C1 + 72:C1 + 76] = np.eye(4, dtype=np.float32)
    wt[53, C1:C1 + 68] = ba
    wt[53, C1 + 76] = 1.0
    # P2: rhs t_b[0:103] -> be: b1-pre + bb via ones
    wt[0:102, C2:C2 + 68] = Wb
    wt[102, C2:C2 + 68] = bb
    # P3: rhs axy.al[0:77] -> X[0:34]: c0 partial + B0
    # meta = (lrelu(m) - lrelu(-m)) / 1.01 folded into W0[136:140]
    wt[0:68, C3:C3 + 34] = W0[0:68]
    wt[68:72, C3:C3 + 34] = W0[136:140] * RM
    wt[72:76, C3:C3 + 34] = -W0[136:140] * RM
    wt[76, C3:C3 + 34] = B0
    # P4: rhs bs[0:68] -> X[0:34]: c0 partial
    wt[0:68, C4:C4 + 34] = W0[68:136]
    # P5: rhs axy.X[0:69] -> X[0:109]: c1 = W1.c0 + B1 (34:68), ones(68)
    wt[0:34, C5 + 34:C5 + 68] = W1
    wt[68, C5 + 34:C5 + 68] = B1
    wt[68, C5 + 68] = 1.0
    # P6: rhs axy.X[0:69] -> Y[0:21]: c2 = W2.c1 + B2 (cols 1:21)
    wt[34:68, C6 + 1:C6 + 21] = W2
    wt[68, C6 + 1:C6 + 21] = B2
    # P7: rhs axy.Y[0:109] -> Y[0:109]: tail chain + biases + ones
    wt[1:21, C7 + 21:C7 + 41] = W3       # c2 -> c3
    wt[21:41, C7 + 41:C7 + 61] = W4      # c3 -> c4
    wt[41:61, C7 + 61:C7 + 81] = W5      # c4 -> c5
    wt[61:81, C7 + 81:C7 + 101] = W6     # c5 -> c6
    wt[81:101, C7 + 101:C7 + 106] = W7   # c6 -> c7
    wt[101:106, C7 + 106:C7 + 108] = W8  # c7 -> c8
    wt[106:108, C7:C7 + 1] = W9          # c8 -> y
    wt[108, C7 + 21:C7 + 41] = B3
    wt[108, C7 + 41:C7 + 61] = B4
    wt[108, C7 + 61:C7 + 81] = B5
    wt[108, C7 + 81:C7 + 101] = B6
    wt[108, C7 + 101:C7 + 106] = B7
    wt[108, C7 + 106:C7 + 108] = B8
    wt[108, C7:C7 + 1] = B9
    wt[108, C7 + 108] = 1.0
    return wt.astype(np.float16)


def _pack_core_inputs(a, b, meta, n_chunks):
    """Pack one core's shard into the fp16 t_a/t_b DMA streams."""
    bc = n_chunks * N
    t_a = np.zeros((KA, bc), np.float16)
    t_a[0:45] = a[:bc].T
    t_a[45:49] = meta[:bc].T
    t_a[49:53] = -meta[:bc].T
    t_a[53] = 1.0
    t_b = np.zeros((KB, bc), np.float16)
    t_b[0:102] = b[:bc].T
    t_b[102] = 1.0
    return t_a, t_b


def build_bass(n_chunks):
    """Build + compile the per-core Bass program (same on all 8 cores)."""
    nc = bacc.Bacc(None, target_bir_lowering=False, debug=False)
    n_steps = n_chunks + PIPE
    n_super = (n_chunks + SUP - 1) // SUP
    NS = SUP * N

    ta_d = nc.dram_tensor("ta", [KA, n_chunks * N], F16,
                          kind="ExternalInput")
    tb_d = nc.dram_tensor("tb", [KB, n_chunks * N], F16,
                          kind="ExternalInput")
    wt_d = nc.dram_tensor("wt", [128, WT_COLS], F16, kind="ExternalInput")
    y_d = nc.dram_tensor("y", [1, n_chunks * N], F16, kind="ExternalOutput")

    with tile.TileContext(nc) as tc:
        with (
            tc.tile_pool(name="const", bufs=1) as constp,
            tc.tile_pool(name="t1p", bufs=3) as t1p,
            tc.tile_pool(name="t2p", bufs=3) as t2p,
            tc.tile_pool(name="axyp", bufs=4) as axyp,
            tc.tile_pool(name="bsp", bufs=4) as bsp,
            tc.tile_pool(name="ps", bufs=2, space=bass.MemorySpace.PSUM) as ps,
            tc.tile_pool(name="psb", bufs=2,
                         space=bass.MemorySpace.PSUM) as psb,
        ):
            wt = constp.tile([128, WT_COLS], F16, tag="wt")
            axy0 = constp.tile([128, 3 * N], F16, tag="axy0")
            nc.sync.dma_start(wt[:], wt_d[:])
            # all-ones init: the warmup lanes only feed garbage chunks
            # (never exported); the ones rows (68/X, 108/Y) must be 1.
            nc.gpsimd.memset(axy0[:], 1.0)

            def w(c0, m):
                return wt[:, c0:c0 + m]

            def chain(*insts):
                for i in range(1, len(insts)):
                    add_dep_helper(insts[i].ins, insts[i - 1].ins,
                                   sync=False, reason="psum acc order")

            t1s, t2s, axys, bss = {}, {}, {}, {}
            axys[-1] = axys[-2] = axys[-3] = axy0

            for t in range(n_steps):
                # ---- input super-DMAs (prefetch 2 supers ahead) ----
                if t % SUP == 0:
                    for s in ([0, 1, 2] if t == 0 else [t // SUP + 2]):
                        if 0 <= s < n_super:
                            t1s[s] = t1p.tile([KA, NS], F16, tag="ta",
                                              name=f"ta_{s}")
                            t2s[s] = t2p.tile([KB, NS], F16, tag="tb",
                                              name=f"tb_{s}")
                            nc.sync.dma_start(
                                t1s[s][:], ta_d[:, s * NS:(s + 1) * NS])
                            nc.sync.dma_start(
                                t2s[s][:], tb_d[:, s * NS:(s + 1) * NS])

                mm = nc.tensor.matmul
                pst = ps.tile([128, 3 * N], F32, tag="ps", name=f"ps_{t}")
                bet = psb.tile([128, N], F32, tag="be", name=f"be_{t}")
                al = pst[:, 0 * N:1 * N]
                X = pst[:, 1 * N:2 * N]
                Y = pst[:, 2 * N:3 * N]
                be = bet[:, 0:N]
                ax2 = axys[t - 2]
                ax3 = axys[t - 3]
                bsp_t = bss.get(t - 2)
                ksl = slice((t % SUP) * N, (t % SUP + 1) * N)

                has_p34 = 2 <= t < n_chunks + 2
                has_p5 = t < n_chunks + 4
                has_p6 = t < n_chunks + 6
                has_p12 = t < n_chunks

                # ---- X bank: P5 (start) <- P3 <- P4 ----
                if has_p5:
                    i5 = mm(X[0:MW], w(C5, MW)[0:128], ax2[0:128, N:2 * N],
                            start=True, stop=not has_p34,
                            tile_position=(0, 0))
                    if has_p34:
                        i3 = mm(X[0:MW], w(C3, MW)[0:128], ax2[0:128, 0:N],
                                start=False, stop=False, tile_position=(0, 0))
                        i4 = mm(X[0:MW], w(C4, MW)[0:128],
                                bsp_t[0:128, 0:N],
                                start=False, stop=True, tile_position=(0, 0))
                        chain(i5, i3, i4)

                # ---- Y bank: P7 (start) <- P6 ----
                i7 = mm(Y[0:MW], w(C7, MW)[0:128], ax2[0:128, 2 * N:3 * N],
                        start=True, stop=not has_p6, tile_position=(0, 0))
                if has_p6:
                    i6 = mm(Y[0:MW], w(C6, MW)[0:128], ax2[0:128, N:2 * N],
                            start=False, stop=True, tile_position=(0, 0))
                    chain(i7, i6)

                # ---- al/be banks: P1, P2a+P2b ----
                if has_p12:
                    s = t // SUP
                    mm(al[0:MW], w(C1, MW)[0:128], t1s[s][0:128, ksl],
                       start=True, stop=True, tile_position=(0, 0))
                    mm(be[0:MW], w(C2, MW)[0:128], t2s[s][0:128, ksl],
                       start=True, stop=True, tile_position=(0, 0))

                # ---- drains ----
                axys[t] = axyp.tile([128, 3 * N], F16, tag="axy",
                                    name=f"axy_{t}")
                lo = 0 if has_p12 else (N if has_p5 else 2 * N)
                nc.scalar.activation(axys[t][0:MW, lo:3 * N],
                                     pst[0:MW, lo:3 * N], PRELU,
                                     bias=0.0, alpha=ALPHA)
                if has_p12:
                    bss[t] = bsp.tile([128, N], F16, tag="bs",
                                      name=f"bs_{t}")
                    nc.vector.tensor_copy(bss[t][0:128, 0:N],
                                          be[0:128])
                    nc.vector.scalar_tensor_tensor(
                        bss[t][0:128, 0:N], bss[t][0:128, 0:N], ALPHA,
                        bss[t][0:128, 0:N], ALU.mult, ALU.max)

                # ---- y out ----
                if t >= PIPE:
                    c = t - PIPE
                    nc.gpsimd.dma_start(
                        y_d[:, c * N:(c + 1) * N],
                        axys[t][0:1, 2 * N:3 * N])

    nc.compile()
    return nc


_NC_CACHE = {}


def _get_nc(n_chunks):
    if n_chunks not in _NC_CACHE:
        _NC_CACHE[n_chunks] = build_bass(n_chunks)
    return _NC_CACHE[n_chunks]


def run_cores(inputs, n_chunks, cores, trace=False, trace_kwargs=None):
    """Pack inputs, run the SPMD kernel on the given cores, return
    (per-core y arrays, BassKernelResults)."""
    a = np.asarray(inputs["a"], np.float32)
    b = np.asarray(inputs["b"], np.float32)
    meta = np.asarray(inputs["meta"], np.float32)
    Ws = [np.asarray(inputs[f"W{i}"], np.float32) for i in range(10)]
    Bs = [np.asarray(inputs[f"B{i}"], np.float32) for i in range(10)]
    wt = _pack_weights(np.asarray(inputs["Wa"], np.float32),
                       np.asarray(inputs["ba"], np.float32),
                       np.asarray(inputs["Wb"], np.float32),
                       np.asarray(inputs["bb"], np.float32), Ws, Bs)
    in_maps = []
    for r in cores:
        sl = slice(r * B_CORE, r * B_CORE + n_chunks * N)
        t_a, t_b = _pack_core_inputs(a[sl], b[sl], meta[sl], n_chunks)
        in_maps.append({"ta": t_a, "tb": t_b, "wt": wt})
    nc = _get_nc(n_chunks)
    kw = dict(trace=trace)
    if trace_kwargs:
        kw.update(trace_kwargs)
    res = run_bass_kernel_spmd(nc, in_maps, list(range(len(cores))), **kw)
    return [res.results[i]["y"].astype(np.float32) for i in range(len(cores))], res


def kernel(**inputs):
    n_chunks = B_CORE // N
    ys, _ = run_cores(inputs, n_chunks, list(range(N_CORES)))
    out = np.empty((B_FULL, 1), np.float32)
    for r in range(N_CORES):
        out[r * B_CORE:(r + 1) * B_CORE, 0] = ys[r][0]
    return out
